# revision 13
# baseline (speedup 1.0000x reference)
"""Trainium2 Bass kernel for nn_AttentionAddition (8-core SPMD).

Sharding: data-parallel over the N (RoI) dimension. Each core owns Q = N/8
queries. K/V for the attention (kp = relu(sem) @ Wqk^T plus dummy row,
vv = comb) are computed shard-wise and exchanged with 2 pipelined AllGather
waves in fp8 (e4m3).

The entire attention path (qp/kp projections, comb, S = qp kp^T, exp,
PV) runs in fp8 with DoubleRow matmuls (2 k-chunks of 128 contracted per
instruction). The epilogue (z1/z2, o3, LayerNorm, FFN) stays bf16 - fp8
there breaches the accuracy budget.

Layout convention on device: activations are kept TRANSPOSED, i.e.
[feature, query] with the feature dim on SBUF partitions (tiles are
[128, n_chunks, q]). In this orientation every Linear of the module maps to
matmul(out, lhsT=W^T chunk, rhs=activation chunk) with a per-partition bias
via the ScalarE activation op, and no activation ever needs a transpose.
The single final transpose back to [query, feature] is done on the PE.
"""

import numpy as np
import ml_dtypes

import concourse.bass as bass
import concourse.tile as tile
from concourse import bacc, mybir
from concourse.masks import make_identity

F32 = mybir.dt.float32
BF16 = mybir.dt.bfloat16
FP8 = mybir.dt.float8e4
AF = mybir.ActivationFunctionType
ALU = mybir.AluOpType
DR = mybir.MatmulPerfMode.DoubleRow

NCORES = 8
N, D, S, C = 8192, 1024, 300, 80
Q = N // NCORES          # queries per core = 1024
DC = D // 128            # feature chunks = 8
QC = Q // 128            # query chunks per core = 8
SCALE = 1.0 / np.sqrt(np.float32(D))  # 1/32


def _mk_ap(base_ap, offset_elems, dims):
    """Raw strided AP on a DRAM tensor. dims = [[step, count], ...]."""
    return bass.AP(
        tensor=base_ap.tensor,
        offset=base_ap.offset + offset_elems,
        ap=[list(d) for d in dims],
    )


class Ctx:
    pass


def build(debug=False, single=False, upto="abc", repeat=1, noag=False,
          o3fp8=None, o12fp8=None):
    import os
    if o3fp8 is None:
        o3fp8 = os.environ.get("O3FP8", "1") == "1"
    if o12fp8 is None:
        o12fp8 = os.environ.get("O12FP8", "1") == "1"
    nc = bacc.Bacc("TRN2", target_bir_lowering=False, debug=False,
                   num_devices=1 if single else NCORES)
    cx = Ctx()
    cx.debug = debug
    cx.single = single
    cx.upto = upto
    cx.noag = noag
    cx.o3fp8 = o3fp8
    cx.o12fp8 = o12fp8
    assert not (o3fp8 and not o12fp8), "o3fp8 needs fp8 o1/o2 tiles"

    def din(name, shape, dt=F32):
        return nc.dram_tensor(name, shape, dt, kind="ExternalInput").ap()

    cx.fpT = din("fpT", [D, Q], BF16)               # feature_pooled shard, transposed
    cx.fpT8 = din("fpT8", [D, Q], FP8)              # same, fp8 (comb lhsT)
    cx.ohT = din("ohT", [C + 1, Q], BF16)           # one-hot(gt_classes), transposed
    cx.cembT = din("cembT", [384, C + 1], BF16)  # [class_embed;bg].T pad, row 300 = 1
    cx.wprojT = din("wprojT", [384, D], BF16)    # w_proj.T pad, row 300 = b_proj
    cx.wcombT8 = din("wcombT8", [2 * D, D], FP8)    # w_comb.T fp8
    cx.wcsemT = din("wcsemT", [D, D], BF16)         # w_comb.T rows 0..D (sem half), bf16
    cx.wqkT16 = din("wqkT16", [D, D], BF16)         # w_qk.T bf16 (for ceP1)
    cx.wcb = din("wcb", [1, D])                     # b_comb
    cx.wqkT8 = din("wqkT8", [D, D], FP8)
    cx.dumT16 = din("dumT16", [D, 16], FP8)         # dummy.T in col 0, rest 0
    cx.w1T = din("w1T", [D, D // 2], BF16)
    cx.w1T8 = din("w1T8", [D, D // 2], FP8)
    cx.b1 = din("b1", [D // 2])
    cx.w2T = din("w2T", [D, D // 2], BF16)
    cx.w2T8 = din("w2T8", [D, D // 2], FP8)
    cx.b2 = din("b2", [D // 2])
    cx.w3T = din("w3T", [2 * D, D], BF16)
    cx.w3T8 = din("w3T8", [2 * D, D], FP8)
    cx.b3 = din("b3", [D])
    cx.wf1T = din("wf1T", [D, D], FP8)
    cx.bf1 = din("bf1", [D])
    cx.wf2T = din("wf2T", [D, D], FP8)
    cx.bf2 = din("bf2", [D])
    cx.lng = din("lng", [D])
    cx.lnb = din("lnb", [D])

    cx.out_d = nc.dram_tensor("out", [Q, D], F32, kind="ExternalOutput").ap()

    cx.dbg = {}
    if debug:
        def dout(name, shape, dt=F32):
            cx.dbg[name] = nc.dram_tensor(name, shape, dt,
                                          kind="ExternalOutput").ap()
        dout("d_ceP1", [C + 1, D], BF16)
        dout("d_ceP2", [C + 1, D], BF16)
        dout("d_comb", [128, QC * D], FP8)
        dout("d_kpT", [128, DC * Q], FP8)
        dout("d_qpT", [128, DC * Q], FP8)
        dout("d_rowsum", [1, Q])
        dout("d_outacc", [128, DC * Q])
        dout("d_oT", [128, DC * Q], BF16)
        dout("d_normT", [128, DC * Q])

    # AllGather buffers, split so the kp gather can fire before comb is done.
    # Wave w = the w-th half of every rank's local keys. kp blocks are
    # kpT[:, w*512:+512] flattened ([1024 d, 512 k] row-major as [512, 1024]);
    # vv blocks are vv[w*512:+512, :] (natural [512 k, 1024 d]).
    cx.bounce_kp = [nc.dram_tensor(f"bkp{w}", [Q // 2, D], FP8,
                                   kind="Internal").ap() for w in range(2)]
    cx.bounce_vv = [nc.dram_tensor(f"bvv{w}", [Q // 2, D], FP8,
                                   kind="Internal").ap() for w in range(2)]
    cx.ag_kp = [nc.dram_tensor(f"agkp{w}", [NCORES * Q // 2, D], FP8,
                               kind="Internal", addr_space="Shared").ap()
                for w in range(2)]
    cx.ag_vv = [nc.dram_tensor(f"agvv{w}", [NCORES * Q // 2, D], FP8,
                               kind="Internal", addr_space="Shared").ap()
                for w in range(2)]

    with tile.TileContext(nc) as tc:
        with tc.tile_pool(name="pp", bufs=1) as pp:
            # all small fp32 constants packed into one 4KB-padded tile
            consts = pp.tile([128, 288], F32)
            cx.ident = consts[:, 0:128]
            make_identity(nc, cx.ident)
            cx.ones_c = consts[:, 128:129]        # ones col (partition reduce)
            nc.vector.memset(cx.ones_c, 1.0)
            cx.eps_t = consts[0:1, 129:130]
            nc.vector.memset(cx.eps_t, 1e-5)
            cx.ones_r = consts[0:1, 130:258]      # ones row (bias mm / bcast)
            nc.vector.memset(cx.ones_r, 1.0)
            cb = pp.tile([128, 130], BF16)
            cx.ones_cb = cb[:, 0:1]
            nc.vector.memset(cx.ones_cb, 1.0)
            cx.ones_rb = cb[0:1, 2:130]
            nc.vector.memset(cx.ones_rb, 1.0)
            # fp8 ones with stride-16 pair layout for DoubleRow rowsum MMs
            c8 = pp.tile([128, 2, 16], FP8)
            nc.vector.memset(c8[:], 1.0)
            cx.ones_pair8 = c8[:, :, 0:1]         # [128, 2, 1], step 16

            def _mark(tag):
                marks.append((tag, sum(1 for _ in nc.all_instructions())))
            marks = []
            nc._phase_marks = marks
            for _rep in range(repeat):
                with tc.tile_pool(name="pq", bufs=1) as pq:
                    cx.pq = pq
                    _mark("A")
                    _phase_a(nc, tc, cx)
                    if cx.upto == "a":
                        nc.gpsimd.dma_start(
                            out=cx.out_d.rearrange("(c p) d -> p c d", p=128),
                            in_=cx.qpT8[:])
                    else:
                        with tc.tile_pool(name="pbc", bufs=1) as pbc:
                            cx.pbc = pbc
                            _mark("B")
                            _phase_b(nc, tc, cx)
                            if cx.upto == "ab":
                                nc.sync.dma_start(
                                    out=cx.out_d
                                    .rearrange("(c p) d -> p c d", p=128),
                                    in_=cx.outn[:])
                            else:
                                _mark("C")
                                _phase_c(nc, tc, cx)
                                if cx.upto != "abc":
                                    nc.sync.dma_start(
                                        out=cx.out_d
                                        .rearrange("(c p) d -> p c d", p=128),
                                        in_=cx.outn[:])
    marks.append(("end", sum(1 for _ in nc.all_instructions())))
    nc.compile()
    return nc


def _phase_a(nc, tc, cx):
    """Projections. The class table is tiny (81 rows), so every per-RoI
    quantity that is a pure function of the class is precomputed on the
    table and GATHERED with a one-hot matmul (contraction 81):
      ceP1 = relu(ce) @ w_qk.T   -> kp  = gather(ceP1)   [relu(sem) = gather(relu(ce))]
      ceP2 = ce @ w_comb[:, :D].T + b_comb -> comb_sem = gather(ceP2)
    Order chosen so attention unblocks ASAP: ceT -> ceP1 -> kp gather
    (+kp AllGather per wave) -> comb (+vv AllGather) -> vis -> qp.
    fp8 DR matmuls keep each stationary operand for 2 moving tiles
    (2 PSUM banks) - LDWEIGHTS then hides under the matmul pair.
    """
    debug, dbg = cx.debug, cx.dbg

    def ship(bounce, agb, bounce_ap, in_ap):
        nc.sync.dma_start(out=bounce_ap, in_=in_ap)
        if cx.noag:
            return
        if cx.single:
            for r in range(NCORES):
                nc.sync.dma_start(
                    out=agb[r * (Q // 2):(r + 1) * (Q // 2), :], in_=bounce)
        else:
            nc.gpsimd.collective_compute(
                "AllGather", ALU.bypass,
                replica_groups=[list(range(NCORES))],
                ins=[bounce], outs=[agb])

    with (
        tc.tile_pool(name="paV", bufs=1) as paV,
        tc.tile_pool(name="paw", bufs=2) as paw,
        tc.tile_pool(name="pap", bufs=5, space="PSUM") as pap,
    ):
        oh_sb = paV.tile([C + 1, Q], BF16)
        nc.sync.dma_start(out=oh_sb[:], in_=cx.ohT)
        # fp half of w_comb.T (rows D..2D), fp8, keyed by output 512-block
        wcqs = {}
        for _ob in range(2):
            _os = slice(_ob * 512, (_ob + 1) * 512)
            wcq = paw.tile([128, DC, 512], FP8, tag="wcq")
            nc.scalar.dma_start(
                out=wcq[:],
                in_=cx.wcombT8[D:, _os].rearrange("(c p) o -> p c o", p=128))
            wcqs[_ob] = wcq

        with tc.tile_pool(name="pa0", bufs=1) as pa0:
            # ceT = (cemb@wproj.T+b).T directly: [d, 81] chunks
            cembT_sb = pa0.tile([128, 3, C + 1], BF16)
            nc.sync.dma_start(out=cembT_sb[:],
                              in_=cx.cembT.rearrange("(c p) n -> p c n", p=128))
            wprojT_sb = pa0.tile([128, 3, D], BF16)
            nc.sync.dma_start(out=wprojT_sb[:],
                              in_=cx.wprojT.rearrange("(c p) d -> p c d", p=128))
            wqk16_sb = pa0.tile([128, DC, D], BF16)
            nc.sync.dma_start(out=wqk16_sb[:],
                              in_=cx.wqkT16.rearrange("(c p) o -> p c o", p=128))
            wcs16_sb = pa0.tile([128, DC, D], BF16)
            nc.scalar.dma_start(out=wcs16_sb[:],
                                in_=cx.wcsemT.rearrange("(c p) o -> p c o", p=128))
            wcb_row = pa0.tile([1, D], F32)
            nc.sync.dma_start(out=wcb_row[:], in_=cx.wcb)
            wcbB = pa0.tile([1, D], BF16)
            nc.scalar.copy(out=wcbB[:], in_=wcb_row[:])

            ceT_sb = pa0.tile([128, DC, C + 1], BF16)
            ceRT_sb = pa0.tile([128, DC, C + 1], BF16)
            for dc in range(DC):
                ce_ps = pap.tile([128, C + 1], F32, tag="ps")
                for sc in range(3):
                    nc.tensor.matmul(ce_ps[:],
                                     wprojT_sb[:, sc, dc * 128:(dc + 1) * 128],
                                     cembT_sb[:, sc, :],
                                     start=(sc == 0), stop=(sc == 2))
                nc.scalar.copy(out=ceT_sb[:, dc, :], in_=ce_ps[:])
                nc.scalar.activation(out=ceRT_sb[:, dc, :], in_=ce_ps[:],
                                     func=AF.Relu)

            # ceP1 = relu(ce) @ w_qk.T  [81, D]
            ceP1_sb = pa0.tile([C + 1, D], BF16)
            for ob in range(2):
                os_ = slice(ob * 512, (ob + 1) * 512)
                p_ps = pap.tile([C + 1, 512], F32, tag="ps")
                for dc in range(DC):
                    nc.tensor.matmul(p_ps[:], ceRT_sb[:, dc, :],
                                     wqk16_sb[:, dc, os_],
                                     start=(dc == 0), stop=(dc == DC - 1))
                nc.scalar.copy(out=ceP1_sb[:, os_], in_=p_ps[:])
            if debug:
                nc.sync.dma_start(out=dbg["d_ceP1"], in_=ceP1_sb[:])

            # kp = gather(ceP1), shipped per wave ASAP
            kp_sb = pa0.tile([128, DC, Q], FP8)
            for w in range(2):
                qs = slice(w * 512, (w + 1) * 512)
                for dc in range(DC):
                    g_ps = pap.tile([128, 512], F32, tag="ps")
                    nc.tensor.matmul(g_ps[:],
                                     ceP1_sb[:, dc * 128:(dc + 1) * 128],
                                     oh_sb[:, qs], start=True, stop=True)
                    if dc % 2 == 0:
                        nc.vector.tensor_copy(kp_sb[:, dc, qs], g_ps[:])
                    else:
                        nc.scalar.copy(out=kp_sb[:, dc, qs], in_=g_ps[:])
                ship(cx.bounce_kp[w], cx.ag_kp[w],
                     _mk_ap(cx.bounce_kp[w], 0,
                            [[4096, 128], [512, 8], [1, 512]]),
                     kp_sb[:, :, qs])
            if debug:
                nc.sync.dma_start(out=dbg["d_kpT"],
                                  in_=kp_sb[:].rearrange("p c q -> p (c q)"))

            # ceP2 = ce @ w_comb[:, :D].T + b_comb  [81, D]
            ceP2_sb = pa0.tile([C + 1, D], BF16)
            for ob in range(2):
                os_ = slice(ob * 512, (ob + 1) * 512)
                p_ps = pap.tile([C + 1, 512], F32, tag="ps")
                for dc in range(DC):
                    nc.tensor.matmul(p_ps[:], ceT_sb[:, dc, :],
                                     wcs16_sb[:, dc, os_],
                                     start=(dc == 0), stop=False)
                nc.tensor.matmul(p_ps[:], cx.ones_rb[:, 0:C + 1],
                                 wcbB[:, os_], start=False, stop=True)
                nc.scalar.copy(out=ceP2_sb[:, os_], in_=p_ps[:])
            if debug:
                nc.sync.dma_start(out=dbg["d_ceP2"], in_=ceP2_sb[:])

            fpT8_sb = pa0.tile([128, DC, Q], FP8)
            nc.sync.dma_start(out=fpT8_sb[:],
                              in_=cx.fpT8.rearrange("(c p) q -> p c q", p=128))
            fpT_sb = pa0.tile([128, DC, Q], BF16)
            nc.scalar.dma_start(out=fpT_sb[:],
                              in_=cx.fpT.rearrange("(c p) q -> p c q", p=128))

            # comb = gather(ceP2) + fp @ w_comb[:, D:].T  (natural [q, d]),
            # shipped per wave. fp8 DR part keeps each fp chunk stationary
            # for both 512-col output halves (2 PSUM banks).
            comb_sb = paV.tile([128, QC, D], FP8, name="comb_sb")
            for w in range(2):
                for qc in range(w * 4, w * 4 + 4):
                    qs = slice(qc * 128, (qc + 1) * 128)
                    cb_ps = [pap.tile([128, 512], F32, tag="ps",
                                      name=f"cbps{qc}_{_ob}")
                             for _ob in range(2)]
                    for ob in range(2):
                        os_ = slice(ob * 512, (ob + 1) * 512)
                        nc.tensor.matmul(cb_ps[ob][:], oh_sb[:, qs],
                                         ceP2_sb[:, os_],
                                         start=True, stop=False)
                    for ic in range(0, DC, 2):
                        for ob in range(2):
                            nc.tensor.matmul(cb_ps[ob][:],
                                             fpT8_sb[:, ic:ic + 2, qs],
                                             wcqs[ob][:, ic:ic + 2, :],
                                             start=False, stop=(ic == DC - 2),
                                             perf_mode=DR)
                    for ob in range(2):
                        os_ = slice(ob * 512, (ob + 1) * 512)
                        if ob == 0:
                            nc.vector.tensor_copy(comb_sb[:, qc, os_],
                                                  cb_ps[ob][:])
                        else:
                            nc.scalar.copy(out=comb_sb[:, qc, os_],
                                           in_=cb_ps[ob][:])
                ship(cx.bounce_vv[w], cx.ag_vv[w],
                     _mk_ap(cx.bounce_vv[w], 0,
                            [[4096, 128], [1024, 4], [1, 1024]]),
                     comb_sb[:, w * 4:(w + 1) * 4, :])
            if debug:
                nc.sync.dma_start(out=dbg["d_comb"],
                                  in_=comb_sb[:].rearrange("p c q -> p (c q)"))

            with tc.tile_pool(name="paK", bufs=1) as paK:
                wqk_sb = paK.tile([128, DC, D], FP8)
                nc.sync.dma_start(out=wqk_sb[:],
                                  in_=cx.wqkT8.rearrange("(c p) o -> p c o", p=128))

                # visT = relu(fpT) bf16 (lives until phase C), fp8 copy
                # (lives until phase C when o3's vis part runs there in fp8)
                cx.visT = visT = cx.pq.tile([128, DC, Q], BF16, name="visT")
                cx.visT8 = visT8 = cx.pq.tile([128, DC, Q], FP8, name="visT8")
                for dc in range(DC):
                    nc.vector.tensor_scalar_max(visT[:, dc, :],
                                                fpT_sb[:, dc, :], 0.0)
                    nc.scalar.activation(out=visT8[:, dc, :], in_=fpT8_sb[:, dc, :],
                                         func=AF.Relu)

                # qp = wqk @ visT (queries unblock attention S matmuls);
                # w innermost so each wqk stationary pair serves both halves
                cx.qpT8 = cx.pq.tile([128, DC, Q], FP8, name="qpT8")
                for oc in range(DC):
                    qk_ps = [pap.tile([128, 512], F32, tag="ps",
                                      name=f"qkps{oc}_{_w}")
                             for _w in range(2)]
                    for ic in range(0, DC, 2):
                        for w in range(2):
                            qs = slice(w * 512, (w + 1) * 512)
                            nc.tensor.matmul(
                                qk_ps[w][:],
                                wqk_sb[:, ic:ic + 2, oc * 128:(oc + 1) * 128],
                                visT8[:, ic:ic + 2, qs],
                                start=(ic == 0), stop=(ic == DC - 2),
                                perf_mode=DR)
                    for w in range(2):
                        qs = slice(w * 512, (w + 1) * 512)
                        if (oc + w) % 2 == 0:
                            nc.vector.tensor_copy(cx.qpT8[:, oc, qs],
                                                  qk_ps[w][:])
                        else:
                            nc.scalar.copy(out=cx.qpT8[:, oc, qs],
                                           in_=qk_ps[w][:])
                if debug:
                    nc.sync.dma_start(out=dbg["d_qpT"],
                                      in_=cx.qpT8[:].rearrange("p c q -> p (c q)"))

                # dummy-key contribution to the softmax denominator (seeds
                # rowsum so phase B opens directly with the S matmuls);
                # qh innermost so the dummy pair stays stationary
                cx.rowsum = rowsum = cx.pq.tile([1, Q], F32, name="rowsum")
                dum_sb = paK.tile([128, DC, 16], FP8)
                nc.sync.dma_start(out=dum_sb[:],
                                  in_=cx.dumT16.rearrange("(c p) o -> p c o",
                                                          p=128))
                sd_ps = [pap.tile([1, 512], F32, tag="ps", name=f"sdps{_q}")
                         for _q in range(2)]
                for dc in range(0, DC, 2):
                    for qh in range(2):
                        qs = slice(qh * 512, (qh + 1) * 512)
                        nc.tensor.matmul(sd_ps[qh][:], dum_sb[:, dc:dc + 2, 0:1],
                                         cx.qpT8[:, dc:dc + 2, qs],
                                         start=(dc == 0), stop=(dc == DC - 2),
                                         perf_mode=DR)
                for qh in range(2):
                    qs = slice(qh * 512, (qh + 1) * 512)
                    nc.scalar.activation(out=rowsum[:, qs], in_=sd_ps[qh][:],
                                         func=AF.Exp, scale=float(SCALE))


def _phase_b(nc, tc, cx):
    """Attention: S^T = kp^T-chunks x qpT, E = exp(S/32), out^T += vv^T E.
    All matmuls fp8 DoubleRow (256-contraction per instruction).

    N=512 structure: per 2048-key superblock j, first compute all 32 E tiles
    (16 key-chunks x 2 query-halves, free dim 512) into one [128,16,512]
    fp8 tile per query half, then do the PV matmuls in two d-half passes per
    query-half so the PV accumulator fits in 4 PSUM banks.
    PSUM: S 2 + rowsum 2 + PV 4 = 8 banks.
    """
    debug, dbg = cx.debug, cx.dbg
    qpT8 = cx.qpT8
    rowsum = cx.rowsum
    out_acc = cx.pbc.tile([128, DC, Q], F32, name="out_acc")
    with (
        tc.tile_pool(name="pb", bufs=1) as pb,
        tc.tile_pool(name="pkv", bufs=10) as pkv,
        tc.tile_pool(name="pe", bufs=6) as pe,
        tc.tile_pool(name="pbo", bufs=1, space="PSUM") as pbo,
        tc.tile_pool(name="pbs", bufs=2, space="PSUM") as pbs,
        tc.tile_pool(name="pbr", bufs=2, space="PSUM") as pbr,
    ):
        recip = pb.tile([1, Q], F32)
        recipb = pb.tile([128, Q], F32)
        for jp in range(2):               # pairs of 2048-key superblocks
            kp_t, vv_t, e_t = [], [], []
            for jj in range(2):
                j = jp * 2 + jj
                w, rr = j // 2, (j % 2) * 4
                for s in range(4):
                    r = rr + s
                    kt = pkv.tile([128, DC, 512], FP8, tag="kp")
                    nc.sync.dma_start(
                        out=kt[:],
                        in_=_mk_ap(cx.ag_kp[w], r * (Q // 2) * D,
                                   [[4096, 128], [512, 8], [1, 512]]))
                    kp_t.append(kt)
                e_t.append(
                    [pe.tile([128, 16, 512], FP8, tag="et",
                             name=f"et{jp}_{jj}_{_qh}") for _qh in range(2)])
            for jj in range(2):
                j = jp * 2 + jj
                w, rr = j // 2, (j % 2) * 4
                for s in range(4):
                    r = rr + s
                    vt = pkv.tile([128, 4, D], FP8, tag="vv")
                    nc.sync.dma_start(
                        out=vt[:],
                        in_=_mk_ap(cx.ag_vv[w], r * (Q // 2) * D,
                                   [[4096, 128], [1024, 4], [1, 1024]]))
                    vv_t.append(vt)

            # S + exp for both superblocks; rowsum accumulated across the pair
            r_ps = [pbr.tile([1, 512], F32, tag="rps", name=f"rps{_qh}")
                    for _qh in range(2)]
            for jj in range(2):
                for sk in range(16):
                    s, kc = jj * 4 + sk // 4, sk % 4
                    s_ps = [pbs.tile([128, 512], F32, tag="sps",
                                     name=f"sps{jj}_{sk}_{_qh}")
                            for _qh in range(2)]
                    for dc in range(0, DC, 2):
                        for qh in range(2):
                            qs = slice(qh * 512, (qh + 1) * 512)
                            nc.tensor.matmul(
                                s_ps[qh][:],
                                kp_t[s][:, dc:dc + 2, kc * 128:(kc + 1) * 128],
                                qpT8[:, dc:dc + 2, qs],
                                start=(dc == 0), stop=(dc == DC - 2),
                                perf_mode=DR)
                    for qh in range(2):
                        nc.scalar.activation(out=e_t[jj][qh][:, sk, :],
                                             in_=s_ps[qh][:],
                                             func=AF.Exp, scale=float(SCALE))
                    if sk % 2 == 1:
                        for qh in range(2):
                            nc.tensor.matmul(r_ps[qh][:], cx.ones_pair8,
                                             e_t[jj][qh][:, sk - 1:sk + 1, :],
                                             start=(jj == 0 and sk == 1),
                                             stop=(jj == 1 and sk == 15),
                                             perf_mode=DR)
            for qh in range(2):
                qs = slice(qh * 512, (qh + 1) * 512)
                nc.vector.tensor_add(rowsum[:, qs], rowsum[:, qs], r_ps[qh][:])
            if jp == 1:
                # rowsum is final: broadcast 1/rowsum now so the normalize
                # can hide under the remaining PV matmuls
                nc.vector.reciprocal(recip[:], rowsum[:])
                for qh in range(2):
                    qs = slice(qh * 512, (qh + 1) * 512)
                    b_ps = pbs.tile([128, 512], F32, tag="sps")
                    nc.tensor.matmul(b_ps[:], cx.ones_r, recip[:, qs],
                                     start=True, stop=True)
                    nc.scalar.copy(out=recipb[:, qs], in_=b_ps[:])

            # PV: accumulate both superblocks in PSUM; d-chunk pairs with
            # qh innermost so each vv stationary pair serves both halves
            # (2 banks per qh x 2 qh = 4 PSUM banks)
            for dg in range(4):
                o_ps = [pbo.tile([128, 2, 512], F32, tag=f"ops{_qh}",
                                 name=f"ops{jp}_{dg}_{_qh}")
                        for _qh in range(2)]
                for jj in range(2):
                    for sk in range(0, 16, 2):
                        s, kc = jj * 4 + sk // 4, sk % 4
                        for d4 in range(2):
                            dc = dg * 2 + d4
                            for qh in range(2):
                                nc.tensor.matmul(
                                    o_ps[qh][:, d4, :],
                                    vv_t[s][:, kc:kc + 2,
                                            dc * 128:(dc + 1) * 128],
                                    e_t[jj][qh][:, sk:sk + 2, :],
                                    start=(jj == 0 and sk == 0),
                                    stop=(jj == 1 and sk == 14),
                                    perf_mode=DR)
                for qh in range(2):
                    qs = slice(qh * 512, (qh + 1) * 512)
                    dc = dg * 2
                    if jp == 0:
                        nc.vector.tensor_copy(out_acc[:, dc:dc + 2, qs],
                                              o_ps[qh][:])
                    else:
                        nc.vector.tensor_add(out_acc[:, dc:dc + 2, qs],
                                             out_acc[:, dc:dc + 2, qs],
                                             o_ps[qh][:])
                        for d4 in range(2):
                            nc.vector.tensor_mul(out_acc[:, dc + d4, qs],
                                                 out_acc[:, dc + d4, qs],
                                                 recipb[:, qs])

        if debug:
            nc.sync.dma_start(out=dbg["d_rowsum"], in_=rowsum[:])
            nc.sync.dma_start(out=dbg["d_outacc"],
                              in_=out_acc[:].rearrange("p c q -> p (c q)"))

        cx.outn = out_acc


def _phase_c(nc, tc, cx):
    """Epilogue: o1/o2 -> o3 -> [LN sums | ffn1 | LN finish] -> ffn2 in
    natural [q, d] orientation with the LayerNorm residual transposed into
    the same PSUM accumulation, relu, and per-chunk store.

    All epilogue weights are preloaded whole (2KB-burst DMA layouts).
    """
    debug, dbg = cx.debug, cx.dbg
    outn = cx.outn
    with (
        tc.tile_pool(name="pcB", bufs=1) as pcB,
        tc.tile_pool(name="pcp", bufs=8, space="PSUM") as pcp,
    ):
        # all per-feature bias vectors packed into one 4KB tile
        bias = pcB.tile([128, 48], F32)
        b1_sb = bias[:, 0:4]
        nc.sync.dma_start(out=b1_sb, in_=cx.b1.rearrange("(c p) -> p c", p=128))
        b2_sb = bias[:, 4:8]
        nc.sync.dma_start(out=b2_sb, in_=cx.b2.rearrange("(c p) -> p c", p=128))
        b3_sb = bias[:, 8:16]
        nc.sync.dma_start(out=b3_sb, in_=cx.b3.rearrange("(c p) -> p c", p=128))
        bf1_sb = bias[:, 16:24]
        nc.sync.dma_start(out=bf1_sb, in_=cx.bf1.rearrange("(c p) -> p c", p=128))
        bf2_sb = bias[:, 24:32]
        nc.sync.dma_start(out=bf2_sb, in_=cx.bf2.rearrange("(c p) -> p c", p=128))
        lnb2_sb = bias[:, 32:40]                 # ln_b + bf2 folded
        nc.sync.dma_start(out=lnb2_sb, in_=cx.lnb.rearrange("(c p) -> p c", p=128))
        nc.vector.tensor_add(lnb2_sb, lnb2_sb, bf2_sb)
        lng_sb = bias[:, 40:48]
        nc.sync.dma_start(out=lng_sb, in_=cx.lng.rearrange("(c p) -> p c", p=128))

        with tc.tile_pool(name="pcOT", bufs=1) as pcOT:
            oT_sb = pcOT.tile([128, DC, Q], BF16)
            oT8_sb = pcOT.tile([128, DC, Q], FP8)

            with tc.tile_pool(name="pcA", bufs=1) as pcA:
                vis2 = cx.visT
                ZDT = FP8 if cx.o12fp8 else BF16
                o1_sb = pcA.tile([128, 4, Q], ZDT)
                o2_sb = pcA.tile([128, 4, Q], ZDT)
                with tc.tile_pool(name="pcZ", bufs=1) as pcZ:
                    if cx.o3fp8:
                        w3t8 = pcA.tile([128, 16, D], FP8)
                        nc.scalar.dma_start(
                            out=w3t8[:],
                            in_=cx.w3T8.rearrange("(c p) o -> p c o", p=128))
                    else:
                        w3c4 = []
                        for i in range(4):
                            w3t = pcA.tile([128, 16, 256], BF16, tag="w3",
                                           bufs=4, name=f"w3c{i}")
                            nc.scalar.dma_start(
                                out=w3t[:],
                                in_=cx.w3T[:, i * 256:(i + 1) * 256]
                                .rearrange("(c p) o -> p c o", p=128))
                            w3c4.append(w3t)
                    if cx.o12fp8:
                        # z1/z2 in fp8, whole-Q tiles; the o1/o2 GEMMs are
                        # fp8 DR with each w chunk stationary for both
                        # query halves (2 PSUM banks)
                        w1_sb = pcZ.tile([128, DC, 512], FP8)
                        nc.scalar.dma_start(
                            out=w1_sb[:],
                            in_=cx.w1T8.rearrange("(c p) o -> p c o", p=128))
                        w2_sb = pcZ.tile([128, DC, 512], FP8)
                        nc.scalar.dma_start(
                            out=w2_sb[:],
                            in_=cx.w2T8.rearrange("(c p) o -> p c o", p=128))
                        z1_sb = pcZ.tile([128, DC, Q], FP8)
                        z2_sb = pcZ.tile([128, DC, Q], FP8)
                        for dc in range(DC):
                            nc.vector.tensor_mul(z1_sb[:, dc, :],
                                                 outn[:, dc, :], vis2[:, dc, :])
                            nc.vector.tensor_sub(z2_sb[:, dc, :],
                                                 vis2[:, dc, :], outn[:, dc, :])
                        for half, (o_sb, wh_sb, bh_sb, z_sb) in enumerate(
                                [(o1_sb, w1_sb, b1_sb, z1_sb),
                                 (o2_sb, w2_sb, b2_sb, z2_sb)]):
                            for oc in range(4):
                                m_ps = [pcp.tile([128, 512], F32, tag="cps",
                                                 name=f"z{half}_{oc}_{_q}")
                                        for _q in range(2)]
                                for ic in range(0, DC, 2):
                                    for qh in range(2):
                                        qs = slice(qh * 512, (qh + 1) * 512)
                                        nc.tensor.matmul(
                                            m_ps[qh][:],
                                            wh_sb[:, ic:ic + 2,
                                                  oc * 128:(oc + 1) * 128],
                                            z_sb[:, ic:ic + 2, qs],
                                            start=(ic == 0),
                                            stop=(ic == DC - 2),
                                            perf_mode=DR)
                                for qh in range(2):
                                    qs = slice(qh * 512, (qh + 1) * 512)
                                    if half == 0:
                                        nc.scalar.activation(
                                            out=o_sb[:, oc, qs], in_=m_ps[qh][:],
                                            func=AF.Relu,
                                            bias=bh_sb[:, oc:oc + 1])
                                    else:
                                        nc.vector.tensor_scalar(
                                            out=o_sb[:, oc, qs], in0=m_ps[qh][:],
                                            scalar1=bh_sb[:, oc:oc + 1],
                                            scalar2=0.0,
                                            op0=ALU.add, op1=ALU.max)
                    else:
                        w1_sb = pcZ.tile([128, DC, 512], BF16)
                        nc.scalar.dma_start(out=w1_sb[:],
                                            in_=cx.w1T.rearrange("(c p) o -> p c o", p=128))
                        w2_sb = pcZ.tile([128, DC, 512], BF16)
                        nc.scalar.dma_start(out=w2_sb[:],
                                            in_=cx.w2T.rearrange("(c p) o -> p c o", p=128))
                        for half, (o_sb, wh_sb, bh_sb) in enumerate(
                                [(o1_sb, w1_sb, b1_sb), (o2_sb, w2_sb, b2_sb)]):
                            for qh in range(Q // 512):
                                qs = slice(qh * 512, (qh + 1) * 512)
                                z_sb = pcZ.tile([128, DC, 512], BF16, tag="z", bufs=2)
                                for dc in range(DC):
                                    if half == 0:
                                        nc.vector.tensor_mul(z_sb[:, dc, :],
                                                             outn[:, dc, qs],
                                                             vis2[:, dc, qs])
                                    else:
                                        nc.vector.tensor_sub(z_sb[:, dc, :],
                                                             vis2[:, dc, qs],
                                                             outn[:, dc, qs])
                                for oc in range(4):
                                    m_ps = pcp.tile([128, 512], F32, tag="cps")
                                    for ic in range(DC):
                                        nc.tensor.matmul(
                                            m_ps[:],
                                            wh_sb[:, ic, oc * 128:(oc + 1) * 128],
                                            z_sb[:, ic, :],
                                            start=(ic == 0), stop=(ic == DC - 1))
                                    if half == 0:
                                        nc.scalar.activation(out=o_sb[:, oc, qs],
                                                             in_=m_ps[:],
                                                             func=AF.Relu,
                                                             bias=bh_sb[:, oc:oc + 1])
                                    else:
                                        nc.vector.tensor_scalar(
                                            out=o_sb[:, oc, qs], in0=m_ps[:],
                                            scalar1=bh_sb[:, oc:oc + 1],
                                            scalar2=0.0,
                                            op0=ALU.add, op1=ALU.max)

                if cx.upto == "abz":
                    return
                # o = w3 @ [o1; o2; vis] + b3  (transposed out [d, q])
                if cx.o3fp8:
                    # fp8 DR, each w3 pair stationary for both query halves
                    vis8 = cx.visT8
                    for oc in range(DC):
                        m_ps = [pcp.tile([128, 512], F32, tag="cps",
                                         name=f"o3_{oc}_{_q}")
                                for _q in range(2)]
                        for pr in range(8):
                            if pr < 2:
                                rhs_t, rc = o1_sb, pr * 2
                            elif pr < 4:
                                rhs_t, rc = o2_sb, (pr - 2) * 2
                            else:
                                rhs_t, rc = vis8, (pr - 4) * 2
                            for qh in range(2):
                                qs = slice(qh * 512, (qh + 1) * 512)
                                nc.tensor.matmul(
                                    m_ps[qh][:],
                                    w3t8[:, 2 * pr:2 * pr + 2,
                                         oc * 128:(oc + 1) * 128],
                                    rhs_t[:, rc:rc + 2, qs],
                                    start=(pr == 0), stop=(pr == 7),
                                    perf_mode=DR)
                        for qh in range(2):
                            qs = slice(qh * 512, (qh + 1) * 512)
                            nc.scalar.activation(out=oT_sb[:, oc, qs],
                                                 in_=m_ps[qh][:],
                                                 func=AF.Identity,
                                                 bias=b3_sb[:, oc:oc + 1])
                            if (oc + qh) % 2 == 0:
                                nc.vector.tensor_copy(oT8_sb[:, oc, qs],
                                                      oT_sb[:, oc, qs])
                            else:
                                nc.scalar.copy(out=oT8_sb[:, oc, qs],
                                               in_=oT_sb[:, oc, qs])
                else:
                    for oc in range(DC):
                        w3t = w3c4[oc // 2]
                        wo = (oc % 2) * 128
                        for qh in range(Q // 512):
                            qs = slice(qh * 512, (qh + 1) * 512)
                            m_ps = pcp.tile([128, 512], F32, tag="cps")
                            for ic in range(16):
                                rhs = (o1_sb[:, ic, qs] if ic < 4 else
                                       o2_sb[:, ic - 4, qs] if ic < 8 else
                                       vis2[:, ic - 8, qs])
                                nc.tensor.matmul(m_ps[:],
                                                 w3t[:, ic, wo:wo + 128],
                                                 rhs,
                                                 start=(ic == 0), stop=(ic == 15))
                            nc.scalar.activation(out=oT_sb[:, oc, qs], in_=m_ps[:],
                                                 func=AF.Identity,
                                                 bias=b3_sb[:, oc:oc + 1])
                            if oc % 2 == 0:
                                nc.vector.tensor_copy(oT8_sb[:, oc, qs],
                                                      oT_sb[:, oc, qs])
                            else:
                                nc.scalar.copy(out=oT8_sb[:, oc, qs],
                                               in_=oT_sb[:, oc, qs])
            if debug:
                nc.sync.dma_start(out=dbg["d_oT"],
                                  in_=oT_sb[:].rearrange("p c q -> p (c q)"))
            if cx.upto == "abo":
                return

            with tc.tile_pool(name="pcN", bufs=1) as pcN:
                # LN sums first (PE), then ffn1 (hides the LN scalar chain),
                # then the LN finish, then ffn2.
                normT_h = [pcN.tile([128, DC, 512], F32, name=f"normT{_h}")
                           for _h in range(2)]
                f1_sb = pcN.tile([128, DC, Q], FP8)
                wf1n = pcN.tile([128, DC, D], FP8)
                nc.scalar.dma_start(
                    out=wf1n[:],
                    in_=cx.wf1T.rearrange("(c p) o -> p c o", p=128))
                wf2n = pcN.tile([128, DC, D], FP8)
                nc.scalar.dma_start(
                    out=wf2n[:],
                    in_=cx.wf2T.rearrange("(c p) o -> p c o", p=128))
                with tc.tile_pool(name="pcL", bufs=2) as pcL:
                    sums = []
                    for qh in range(Q // 512):
                        qs = slice(qh * 512, (qh + 1) * 512)
                        sum_ps = pcp.tile([1, 512], F32, tag="cps",
                                          name=f"sum{qh}")
                        ssq_ps = pcp.tile([1, 512], F32, tag="cps",
                                          name=f"ssq{qh}")
                        for dc in range(DC):
                            nc.tensor.matmul(sum_ps[:], cx.ones_cb,
                                             oT_sb[:, dc, qs],
                                             start=(dc == 0), stop=(dc == DC - 1))
                            sq_t = pcL.tile([128, 512], BF16, tag="sq")
                            nc.scalar.activation(out=sq_t[:], in_=oT_sb[:, dc, qs],
                                                 func=AF.Square)
                            nc.tensor.matmul(ssq_ps[:], cx.ones_cb, sq_t[:],
                                             start=(dc == 0), stop=(dc == DC - 1))
                        st = pcL.tile([1, 3, 512], F32, tag="st")
                        slot_a, slot_b, slot_c = (st[:, i, :] for i in range(3))
                        nc.scalar.mul(out=slot_a, in_=sum_ps[:], mul=1.0 / D)  # mu
                        nc.scalar.mul(out=slot_b, in_=ssq_ps[:], mul=1.0 / D)  # E[x^2]
                        nc.vector.tensor_mul(slot_c, slot_a, slot_a)    # mu^2
                        nc.vector.tensor_sub(slot_b, slot_b, slot_c)    # var
                        nc.scalar.activation(out=slot_b, in_=slot_b, func=AF.Sqrt,
                                             bias=cx.eps_t)             # sd
                        nc.vector.reciprocal(slot_c, slot_b)            # c1 = rstd
                        nc.vector.tensor_mul(slot_a, slot_a, slot_c)    # c0 = mu*rstd
                        sums.append((slot_a, slot_c))

                    def ffn1_chunk(ocs):
                        # qh innermost: each wf1 pair stays stationary for
                        # both query halves (2 PSUM banks)
                        for oc in ocs:
                            m_ps = [pcp.tile([128, 512], F32, tag="cps",
                                             name=f"f1_{oc}_{_q}")
                                    for _q in range(2)]
                            for ic in range(0, DC, 2):
                                for qh in range(2):
                                    qs = slice(qh * 512, (qh + 1) * 512)
                                    nc.tensor.matmul(
                                        m_ps[qh][:],
                                        wf1n[:, ic:ic + 2, oc * 128:(oc + 1) * 128],
                                        oT8_sb[:, ic:ic + 2, qs],
                                        start=(ic == 0),
                                        stop=(ic == DC - 2),
                                        perf_mode=DR)
                            for qh in range(2):
                                qs = slice(qh * 512, (qh + 1) * 512)
                                nc.scalar.activation(
                                    out=f1_sb[:, oc, qs], in_=m_ps[qh][:],
                                    func=AF.Relu,
                                    bias=bf1_sb[:, oc:oc + 1])

                    def ln_finish(qh):
                        qs = slice(qh * 512, (qh + 1) * 512)
                        slot_a, slot_c = sums[qh]
                        c1b = pcL.tile([128, 512], F32, tag="c1b")
                        c0b = pcL.tile([128, 512], F32, tag="c0b")
                        for src, dst in [(slot_c, c1b), (slot_a, c0b)]:
                            bb_ps = pcp.tile([128, 512], F32, tag="cps")
                            nc.tensor.matmul(bb_ps[:], cx.ones_r, src,
                                             start=True, stop=True)
                            nc.scalar.copy(out=dst[:], in_=bb_ps[:])
                        for dc in range(DC):
                            tmp = pcL.tile([128, 512], F32, tag="lnt")
                            nc.vector.tensor_mul(tmp[:], oT_sb[:, dc, qs],
                                                 c1b[:])
                            nc.vector.tensor_sub(tmp[:], tmp[:], c0b[:])
                            nc.vector.tensor_scalar(
                                out=normT_h[qh][:, dc, :], in0=tmp[:],
                                scalar1=lng_sb[:, dc:dc + 1],
                                scalar2=lnb2_sb[:, dc:dc + 1],
                                op0=ALU.mult, op1=ALU.add)

                    # ln_finish(0) first: its DVE/Pool normalize overlaps
                    # the ffn1 matmuls; ln_finish(1) overlaps ffn2 qc 0-3.
                    ln_finish(0)
                    ffn1_chunk(range(0, 8))
                    ln_finish(1)
                if debug:
                    for _h in range(2):
                        nc.sync.dma_start(
                            out=_mk_ap(dbg["d_normT"], _h * 512,
                                       [[DC * Q, 128], [Q, DC], [1, 512]]),
                            in_=normT_h[_h][:])
                if cx.upto in ("abl", "abf1"):
                    return

                # FFN layer 2 in natural [q, d] orientation; the LayerNorm
                # residual is transposed into the same PSUM accumulation.
                with tc.tile_pool(name="pcM2", bufs=4) as pcM2:
                    if cx.upto == "abf2":
                        return
                    # dh innermost: each f1 pair stays stationary for both
                    # 512-col output halves (2 PSUM banks)
                    for qc in range(QC):
                        qh = qc // 4
                        m_ps = [pcp.tile([128, 512], F32, tag="cps",
                                         name=f"f2_{qc}_{_d}")
                                for _d in range(2)]
                        for ic in range(0, DC, 2):
                            for dh in range(2):
                                ds_ = slice(dh * 512, (dh + 1) * 512)
                                nc.tensor.matmul(
                                    m_ps[dh][:],
                                    f1_sb[:, ic:ic + 2, qc * 128:(qc + 1) * 128],
                                    wf2n[:, ic:ic + 2, ds_],
                                    start=(ic == 0), stop=False,
                                    perf_mode=DR)
                        qo = (qc % 4) * 128
                        for dh in range(2):
                            ds_ = slice(dh * 512, (dh + 1) * 512)
                            for d4 in range(4):
                                dc = dh * 4 + d4
                                nc.tensor.matmul(
                                    m_ps[dh][:, d4 * 128:(d4 + 1) * 128],
                                    normT_h[qh][:, dc, qo:qo + 128],
                                    cx.ident,
                                    is_transpose=True,
                                    start=False, stop=(d4 == 3),
                                    skip_group_check=True)
                            onat = pcM2.tile([128, 512], F32, tag="onat")
                            nc.scalar.activation(out=onat[:], in_=m_ps[dh][:],
                                                 func=AF.Relu)
                            nc.sync.dma_start(
                                out=cx.out_d[qc * 128:(qc + 1) * 128, ds_],
                                in_=onat[:])



# ---------------------------------------------------------------------------
# Host side
# ---------------------------------------------------------------------------

_CACHE = {}
E4 = ml_dtypes.float8_e4m3


def _to8(x):
    return np.clip(np.asarray(x, np.float32), -240, 240).astype(E4)


def _prep_in_maps(inputs):
    f32 = np.float32
    fp = np.asarray(inputs["feature_pooled"], f32)
    gt = np.asarray(inputs["gt_classes"]).astype(np.int64)
    ce = np.asarray(inputs["class_embed"], f32)
    bg = np.asarray(inputs["bg_embed"], f32)
    w_proj = np.asarray(inputs["w_proj"], f32)
    b_proj = np.asarray(inputs["b_proj"], f32)
    w_comb = np.asarray(inputs["w_comb"], f32)
    b_comb = np.asarray(inputs["b_comb"], f32)
    w_qk = np.asarray(inputs["w_qk"], f32)
    dummy = np.asarray(inputs["dummy"], f32)

    cembT = np.zeros((384, C + 1), f32)
    cembT[:S] = np.concatenate([ce, bg], 0).T
    cembT[S] = 1.0
    cembT = cembT.astype(ml_dtypes.bfloat16)
    wprojT = np.zeros((384, D), f32)
    wprojT[:S] = w_proj.T
    wprojT[S] = b_proj
    wprojT = wprojT.astype(ml_dtypes.bfloat16)
    dumT16 = np.zeros((D, 16), f32)
    dumT16[:, 0] = dummy[0]

    w1T = np.ascontiguousarray(np.asarray(inputs["w1"], f32).T)
    w2T = np.ascontiguousarray(np.asarray(inputs["w2"], f32).T)
    w3T = np.ascontiguousarray(np.asarray(inputs["w3"], f32).T)
    shared = {
        "cembT": cembT,
        "wprojT": wprojT,
        "wcombT8": _to8(np.ascontiguousarray(w_comb.T)),
        "wcsemT": np.ascontiguousarray(w_comb.T[:1024]).astype(ml_dtypes.bfloat16),
        "wqkT16": np.ascontiguousarray(w_qk.T).astype(ml_dtypes.bfloat16),
        "wcb": b_comb[None, :].astype(f32),
        "wqkT8": _to8(np.ascontiguousarray(w_qk.T)),
        "dumT16": _to8(dumT16),
        "w1T": w1T.astype(ml_dtypes.bfloat16),
        "w1T8": _to8(w1T),
        "b1": np.asarray(inputs["b1"], f32),
        "w2T": w2T.astype(ml_dtypes.bfloat16),
        "w2T8": _to8(w2T),
        "b2": np.asarray(inputs["b2"], f32),
        "w3T": w3T.astype(ml_dtypes.bfloat16),
        "w3T8": _to8(w3T),
        "b3": np.asarray(inputs["b3"], f32),
        "wf1T": _to8(np.ascontiguousarray(np.asarray(inputs["wf1"], f32).T)),
        "bf1": np.asarray(inputs["bf1"], f32),
        "wf2T": _to8(np.ascontiguousarray(np.asarray(inputs["wf2"], f32).T)),
        "bf2": np.asarray(inputs["bf2"], f32),
        "lng": np.asarray(inputs["ln_g"], f32),
        "lnb": np.asarray(inputs["ln_b"], f32),
    }
    in_maps = []
    for c in range(NCORES):
        qs = slice(c * Q, (c + 1) * Q)
        oh = np.zeros((C + 1, Q), ml_dtypes.bfloat16)
        oh[gt[qs], np.arange(Q)] = 1.0
        m = dict(shared)
        fpTc = np.ascontiguousarray(fp[qs].T)
        m["fpT"] = fpTc.astype(ml_dtypes.bfloat16)
        m["fpT8"] = _to8(fpTc)
        m["ohT"] = oh
        in_maps.append(m)
    return in_maps


def get_nc(debug=False):
    key = ("nc", debug)
    if key not in _CACHE:
        _CACHE[key] = build(debug=debug)
    return _CACHE[key]


def kernel(**inputs):
    from concourse import bass_utils
    try:
        # persistent XLA/PJRT compile cache so repeat invocations (fresh
        # processes included) skip the NEFF compile
        import jax
        jax.config.update("jax_compilation_cache_dir", "/tmp/jax_neff_cache")
        jax.config.update("jax_persistent_cache_min_compile_time_secs", 1.0)
        jax.config.update("jax_persistent_cache_min_entry_size_bytes", 0)
    except Exception:
        pass
    nc = get_nc(debug=False)
    in_maps = _prep_in_maps(inputs)
    res = bass_utils.run_bass_kernel_spmd(
        nc, in_maps, core_ids=list(range(NCORES)), trace=False)
    return np.concatenate([res.results[c]["out"] for c in range(NCORES)], axis=0)

